# revision 7
# baseline (speedup 1.0000x reference)
"""Causal self-attention Trainium2 kernel (8 NeuronCores).

Sharding: core = (batch b in {0,1}, head-group hg in {0..3}); each core owns
4 of the 16 heads (256 of the 1024 q/k/v dims) for one batch element.
Data parallel over batch, tensor parallel over heads; W_o is row-parallel so
each core emits a partial output that the host sums (+ b_o) at gather time.

Device dataflow (per core), everything in "transposed" layout so the
contraction dim always sits on SBUF partitions:
  xT [1024,2048], wT weights pre-transposed on host.
  QT/KT [d=256, s=2048]  (d on partitions, 2 blocks of 128)
  V natural [s, d] with a ones-column appended per head so the A@V matmul
  also produces the softmax denominator (row 64 of the PSUM tile).
  Scores are computed transposed: ST[k, q] = KT_chunk.T @ QT, masked causally
  (additive -1e30), exp'ed on ACT (no max subtraction needed: |scores|<~3),
  then A@V: outT[d, q] = V_chunk.T @ AexpT accumulated over k chunks.
  Normalization: reciprocal of denom row, gpsimd partition_broadcast, one
  DVE multiply per head. Final: yT[m, s] = WoT.T @ outT (partial over this
  core's 256 d dims).
"""

import sys

for _p in ("/opt/trn_rl_repo",):
    if _p not in sys.path:
        sys.path.insert(0, _p)

import numpy as np

import concourse.bass as bass
import concourse.bacc as bacc
import concourse.mybir as mybir
from concourse import tile
from concourse.bass_utils import run_bass_kernel_spmd

P = 128
S = 2048  # sequence length
D = 1024  # d_model
DG = 256  # dims per head-group (4 heads x 64)
DH = 64   # head dim
NHG = 4   # heads per core
EC = D // P   # 8 contraction chunks over d_model
KC = S // P   # 16 key chunks
QTW = 512     # q tile width
NQT = S // QTW  # 4 q tiles
F32 = mybir.dt.float32
F32R = mybir.dt.float32r
NEG = -1.0e30
AF = mybir.ActivationFunctionType

LAST_EXEC_NS = None
LAST_RESULTS = None


def _r(ap):
    return ap  # tiles are declared float32r already


def _emit(tc, aps):
    nc = tc.nc
    xt_d, wqt_d, wkt_d, wvt_d, wot_d, bq_d, bk_d, bv_d, yt_d = aps

    with (
        tc.tile_pool(name="const", bufs=1) as constp,
        tc.tile_pool(name="wpool", bufs=1) as wp,
        tc.tile_pool(name="xpool", bufs=1) as xp,
        tc.tile_pool(name="qkvp", bufs=1) as qkvp,
        tc.tile_pool(name="aep", bufs=5) as aep,
        tc.tile_pool(name="outp", bufs=1) as outp,
        tc.tile_pool(name="normp", bufs=2) as normp,
        tc.tile_pool(name="stagep", bufs=3) as stagep,
        tc.tile_pool(name="psum_st", bufs=3, space="PSUM") as psum_st,
        tc.tile_pool(name="psum_mm", bufs=5, space="PSUM") as psum_mm,
    ):
        # ---- persistent SBUF tensors ----
        mask_wide = constp.tile([P, 896], F32, name="mask_wide")
        bq_sb = constp.tile([P, 2], F32, name="bq_sb")
        bqs_sb = constp.tile([P, 2], F32, name="bqs_sb")
        bk_sb = constp.tile([P, 2], F32, name="bk_sb")
        bv1_sb = constp.tile([1, DG], F32, name="bv1_sb")
        bvb_sb = constp.tile([P, DG], F32, name="bvb_sb")

        wqt_sb = wp.tile([P, EC, DG], F32R, name="wqt_sb")
        wkt_sb = wp.tile([P, EC, DG], F32R, name="wkt_sb")
        wvt_sb = wp.tile([P, EC, DG], F32R, name="wvt_sb")
        wot_sb = wp.tile([P, 2, D], F32R, name="wot_sb")

        xt_sb = xp.tile([P, EC, S], F32R, name="xt_sb")

        qt_sb = qkvp.tile([P, 2, S], F32R, name="qt_sb")
        kt_sb = qkvp.tile([P, 2, S], F32R, name="kt_sb")
        v_sb = qkvp.tile([P, KC, NHG, DH + 1], F32R, name="v_sb")

        outt_sb = outp.tile([P, 2, S], F32R, name="outt_sb")

        # ---- constants ----
        for c in range(2):
            nc.sync.dma_start(bq_sb[:, c : c + 1], bq_d[c * P : (c + 1) * P, :])
            nc.sync.dma_start(bk_sb[:, c : c + 1], bk_d[c * P : (c + 1) * P, :])
        nc.scalar.mul(bqs_sb[:, :], bq_sb[:, :], 0.125)
        nc.sync.dma_start(bv1_sb[:, :], bv_d[:, :])
        nc.gpsimd.partition_broadcast(bvb_sb[:, :], bv1_sb[:, :], channels=P)
        # causal mask, shared across diagonal offsets: mask_wide[x, y] = 0 if
        # y - x >= 384 else -1e30; slice [384-128j : 896-128j] gives the mask
        # for a diagonal chunk with k0 = q0 + 128j.
        nc.gpsimd.memset(mask_wide[:, :], 0.0)
        nc.gpsimd.affine_select(
            out=mask_wide[:, :],
            in_=mask_wide[:, :],
            compare_op=mybir.AluOpType.is_ge,
            fill=NEG,
            base=-384,
            pattern=[[1, 896]],
            channel_multiplier=-1,
        )
        # ones column in V for the fused softmax denominator
        nc.vector.memset(v_sb[:, :, :, DH : DH + 1].bitcast(F32), 1.0)

        # ---- input DMAs ----
        for ec in range(EC):
            nc.sync.dma_start(xt_sb[:, ec, :], xt_d[ec * P : (ec + 1) * P, :])
            nc.sync.dma_start(wqt_sb[:, ec, :], wqt_d[ec * P : (ec + 1) * P, :])
            nc.sync.dma_start(wkt_sb[:, ec, :], wkt_d[ec * P : (ec + 1) * P, :])
            nc.sync.dma_start(wvt_sb[:, ec, :], wvt_d[ec * P : (ec + 1) * P, :])
        for dc in range(2):
            nc.sync.dma_start(wot_sb[:, dc, :], wot_d[dc * P : (dc + 1) * P, :])

        # ---- QKV projections ----
        # QT/KT: [d-block 128, s 512] = sum_e wT_chunk.T @ xT_chunk
        for db in range(2):
            for t in range(NQT):
                pq = psum_mm.tile([P, QTW], F32, name="pq", tag="mm")
                for ec in range(EC):
                    nc.tensor.matmul(
                        pq[:, :],
                        _r(wqt_sb[:, ec, db * P : (db + 1) * P]),
                        _r(xt_sb[:, ec, t * QTW : (t + 1) * QTW]),
                        start=(ec == 0),
                        stop=(ec == EC - 1),
                    )
                # Q scaled by 1/sqrt(dh)=0.125 here (bias pre-scaled too)
                nc.scalar.activation(
                    qt_sb[:, db, t * QTW : (t + 1) * QTW],
                    pq[:, :],
                    AF.Identity,
                    bias=bqs_sb[:, db : db + 1],
                    scale=0.125,
                )
                pk = psum_mm.tile([P, QTW], F32, name="pk", tag="mm")
                for ec in range(EC):
                    nc.tensor.matmul(
                        pk[:, :],
                        _r(wkt_sb[:, ec, db * P : (db + 1) * P]),
                        _r(xt_sb[:, ec, t * QTW : (t + 1) * QTW]),
                        start=(ec == 0),
                        stop=(ec == EC - 1),
                    )
                nc.scalar.activation(
                    kt_sb[:, db, t * QTW : (t + 1) * QTW],
                    pk[:, :],
                    AF.Identity,
                    bias=bk_sb[:, db : db + 1],
                    scale=1.0,
                )
        # V natural [s-chunk 128, d 256]: xT_chunk (stationary) vs wvT (moving)
        for sc in range(KC):
            pv = psum_mm.tile([P, QTW], F32, name="pv", tag="mm")
            for ec in range(EC):
                nc.tensor.matmul(
                    pv[:, :DG],
                    _r(xt_sb[:, ec, sc * P : (sc + 1) * P]),
                    _r(wvt_sb[:, ec, :]),
                    start=(ec == 0),
                    stop=(ec == EC - 1),
                )
            nc.vector.tensor_add(
                v_sb[:, sc, :, 0:DH],
                pv[:, :DG].rearrange("p (h d) -> p h d", h=NHG),
                bvb_sb[:, :].rearrange("p (h d) -> p h d", h=NHG),
            )

        # ---- attention (per head pair, per q tile) ----
        for m in range(2):
            for t in range(NQT):
                ncnk = 4 * t + 4  # causal: k chunks 0 .. 4t+3
                avs = []
                for hh in range(2):
                    h = 2 * m + hh
                    db, po = divmod(h, 2)
                    qrhs = _r(qt_sb[po * DH : (po + 1) * DH, db, t * QTW : (t + 1) * QTW])

                    def emit_st(c):
                        stp = psum_st.tile([P, QTW], F32, name="stp", tag="st")
                        nc.tensor.matmul(
                            stp[:, :],
                            _r(kt_sb[po * DH : (po + 1) * DH, db, c * P : (c + 1) * P]),
                            qrhs,
                            start=True,
                            stop=True,
                        )
                        return stp

                    av = psum_mm.tile([P, QTW], F32, name="av", tag="mm")
                    stp = emit_st(0)
                    for c in range(ncnk):
                        if c >= 4 * t:
                            off = 384 - 128 * (c - 4 * t)
                            nc.vector.tensor_add(
                                stp[:, :], stp[:, :], mask_wide[:, off : off + QTW]
                            )
                        ae = aep.tile([P, QTW], F32R, name="ae", tag="ae")
                        nc.scalar.activation(ae[:, :], stp[:, :], AF.Exp)
                        if c + 1 < ncnk:
                            stp = emit_st(c + 1)  # keep PE busy during exp
                        nc.tensor.matmul(
                            av[0 : DH + 1, :],
                            _r(v_sb[:, c, h, :]),
                            _r(ae[:, :]),
                            start=(c == 0),
                            stop=(c == ncnk - 1),
                        )
                    avs.append(av)

                # normalize the pair: rows 0..63 data, row 64 denominator
                r0 = normp.tile([1, QTW], F32, name="r0", tag="recip")
                nc.vector.reciprocal(r0[:, :], avs[0][DH : DH + 1, :])
                r1 = normp.tile([1, QTW], F32, name="r1", tag="recip")
                nc.vector.reciprocal(r1[:, :], avs[1][DH : DH + 1, :])
                b0 = normp.tile([DH, QTW], F32, name="b0", tag="bc")
                nc.gpsimd.partition_broadcast(b0[:, :], r0[:, :], channels=DH)
                b1 = normp.tile([DH, QTW], F32, name="b1", tag="bc")
                nc.gpsimd.partition_broadcast(b1[:, :], r1[:, :], channels=DH)
                nc.vector.tensor_mul(
                    outt_sb[0:DH, m, t * QTW : (t + 1) * QTW],
                    avs[0][0:DH, :],
                    b0[:, :],
                )
                # odd head lands on partitions 64..127: scale into an SBUF
                # temp at base 0, then partition-shifting SBUF->SBUF DMA
                odd = normp.tile([DH, QTW], F32R, name="odd", tag="odd")
                nc.vector.tensor_mul(odd[:, :], avs[1][0:DH, :], b1[:, :])
                nc.sync.dma_start(
                    outt_sb[DH:P, m, t * QTW : (t + 1) * QTW], odd[:, :]
                )

        # ---- output projection: yT[m, s] += WoT_chunk.T @ outT_chunk ----
        for mc in range(8):
            pys = [
                psum_mm.tile([P, QTW], F32, name=f"py{st4}", tag="mm")
                for st4 in range(NQT)
            ]
            for dc in range(2):
                for st4 in range(NQT):
                    nc.tensor.matmul(
                        pys[st4][:, :],
                        _r(wot_sb[:, dc, mc * P : (mc + 1) * P]),
                        _r(outt_sb[:, dc, st4 * QTW : (st4 + 1) * QTW]),
                        start=(dc == 0),
                        stop=(dc == 1),
                    )
            for st4 in range(NQT):
                sg = stagep.tile([P, QTW], F32, name="sg", tag="yst")
                nc.scalar.copy(sg[:, :], pys[st4][:, :])
                nc.sync.dma_start(
                    yt_d[mc * P : (mc + 1) * P, st4 * QTW : (st4 + 1) * QTW],
                    sg[:, :],
                )


_NC_CACHE = None


def build_nc():
    global _NC_CACHE
    if _NC_CACHE is not None:
        return _NC_CACHE
    nc = bacc.Bacc("TRN2")
    xt = nc.dram_tensor("xt", [D, S], F32R, kind="ExternalInput")
    wqt = nc.dram_tensor("wqt", [D, DG], F32R, kind="ExternalInput")
    wkt = nc.dram_tensor("wkt", [D, DG], F32R, kind="ExternalInput")
    wvt = nc.dram_tensor("wvt", [D, DG], F32R, kind="ExternalInput")
    wot = nc.dram_tensor("wot", [DG, D], F32R, kind="ExternalInput")
    bq = nc.dram_tensor("bq", [DG, 1], F32, kind="ExternalInput")
    bk = nc.dram_tensor("bk", [DG, 1], F32, kind="ExternalInput")
    bv = nc.dram_tensor("bv", [1, DG], F32, kind="ExternalInput")
    yt = nc.dram_tensor("yt", [D, S], F32, kind="ExternalOutput")
    aps = tuple(h.ap() for h in (xt, wqt, wkt, wvt, wot, bq, bk, bv, yt))
    with tile.TileContext(nc) as tc:
        _emit(tc, aps)
    nc.finalize()
    _NC_CACHE = nc
    return nc


def make_in_maps(x, W_q, b_q, W_k, b_k, W_v, b_v, W_o):
    in_maps = []
    for core in range(8):
        b, hg = divmod(core, 4)
        sl = slice(hg * DG, (hg + 1) * DG)
        in_maps.append(
            {
                "xt": np.ascontiguousarray(np.asarray(x)[b].T, dtype=np.float32),
                "wqt": np.ascontiguousarray(np.asarray(W_q)[sl, :].T, dtype=np.float32),
                "wkt": np.ascontiguousarray(np.asarray(W_k)[sl, :].T, dtype=np.float32),
                "wvt": np.ascontiguousarray(np.asarray(W_v)[sl, :].T, dtype=np.float32),
                "wot": np.ascontiguousarray(np.asarray(W_o)[:, sl].T, dtype=np.float32),
                "bq": np.ascontiguousarray(
                    np.asarray(b_q)[sl].reshape(DG, 1), dtype=np.float32
                ),
                "bk": np.ascontiguousarray(
                    np.asarray(b_k)[sl].reshape(DG, 1), dtype=np.float32
                ),
                "bv": np.ascontiguousarray(
                    np.asarray(b_v)[sl].reshape(1, DG), dtype=np.float32
                ),
            }
        )
    return in_maps


def kernel(x, W_q, b_q, W_k, b_k, W_v, b_v, W_o, b_o, _trace=False):
    global LAST_EXEC_NS, LAST_RESULTS
    nc = build_nc()
    in_maps = make_in_maps(x, W_q, b_q, W_k, b_k, W_v, b_v, W_o)
    kw = {"trace": True} if _trace else {}
    res = run_bass_kernel_spmd(nc, in_maps, core_ids=list(range(8)), **kw)
    LAST_EXEC_NS = res.exec_time_ns
    LAST_RESULTS = res
    b_o = np.asarray(b_o, dtype=np.float32)
    out = np.empty((2, S, D), np.float32)
    for b in range(2):
        ysum = (
            res.results[4 * b]["yt"]
            + res.results[4 * b + 1]["yt"]
            + res.results[4 * b + 2]["yt"]
            + res.results[4 * b + 3]["yt"]
        )
        out[b] = ysum.T + b_o
    return out
